# revision 1
# baseline (speedup 1.0000x reference)
"""LSTMCell (B=16384, I=H=512) on 8 Trainium2 NeuronCores.

Strategy: data-parallel over the batch (2048 rows/core). Each core computes
gatesT = W @ [x;h]T in transposed layout (gate dim on partitions, batch on the
free dim) so that:
  - the contraction dim (I+H) lands on SBUF partitions for both matmul
    operands with zero on-chip transposes (inputs are pre-transposed on the
    host while sharding),
  - the gate bias is a per-partition vector, applied for free by the ScalarE
    activation instruction,
  - fp32 data runs through the PE at bf16 rate via the float32r dtype
    (moving free dim 512 >= 256).
The stacked gate dim is permuted on the host so each 128-row h-block's four
gate tiles (i, f, g, o) are contiguous in the weight matrix, letting weights
stream in [128, 512] chunks in exactly the order the PE consumes them.
Elementwise LSTM tail (sigmoid/tanh/mul/add) runs on ScalarE + VectorE
overlapped with the matmuls; outputs are stored transposed and un-transposed
on the host.
"""

import numpy as np
from contextlib import ExitStack

_B, _I, _H = 16384, 512, 512
_NC = 8
_BL = _B // _NC          # 2048 batch rows per core
_G = 4 * _H              # 2048 stacked gate dim
_K = _I + _H             # 1024 contraction dim
_BCH = 512               # batch chunk (PSUM bank free size)
_NB = _BL // _BCH        # 4 batch chunks
_NJ = _H // 128          # 4 h-blocks of 128
_NK = _K // 128          # 8 k-chunks of 128
_NT = 4                  # gates (i, f, g, o)

_cache = {}


def _build(reps=1):
    from concourse import bacc
    import concourse.mybir as mybir
    import concourse.tile as tile

    f32 = mybir.dt.float32
    f32r = mybir.dt.float32r
    AF = mybir.ActivationFunctionType

    nc = bacc.Bacc("TRN2", target_bir_lowering=False, debug=False,
                   num_devices=_NC)
    xT = nc.declare_dram_parameter("xT", [_I, _BL], f32r, isOutput=False)
    hT = nc.declare_dram_parameter("hT", [_H, _BL], f32r, isOutput=False)
    cT = nc.declare_dram_parameter("cT", [_H, _BL], f32, isOutput=False)
    # gate dim pre-permuted on host: column block j*512..j*512+512 holds the
    # (i, f, g, o) tiles for h-block j, each 128 wide.
    wT = nc.declare_dram_parameter("wT", [_K, _G], f32r, isOutput=False)
    b2 = nc.declare_dram_parameter("b2", [128, _G // 128], f32, isOutput=False)
    hoT = nc.declare_dram_parameter("hoT", [_H, _BL], f32, isOutput=True)
    coT = nc.declare_dram_parameter("coT", [_H, _BL], f32, isOutput=True)

    with ExitStack() as ctx:
        tc = ctx.enter_context(tile.TileContext(nc))
        wp = ctx.enter_context(tc.tile_pool(name="w", bufs=1))
        xp = ctx.enter_context(tc.tile_pool(name="xh", bufs=1))
        bp = ctx.enter_context(tc.tile_pool(name="bias", bufs=1))
        cp = ctx.enter_context(tc.tile_pool(name="cin", bufs=3))
        ap = ctx.enter_context(tc.tile_pool(name="act", bufs=2))
        op = ctx.enter_context(tc.tile_pool(name="out", bufs=2))
        pp = ctx.enter_context(tc.tile_pool(name="ps", bufs=2, space="PSUM"))

        def body(_iv=None):
            bias_sb = bp.tile([128, _G // 128], f32, tag="bias")
            nc.sync.dma_start(out=bias_sb[:], in_=b2[:])

            # Weight tiles [128k, 512g] per (k, j); activation tiles
            # [128k, 512b] per (k, bc). Issued in the order the PE consumes
            # them: everything group (bc=0, j=0) needs first, then j-blocks,
            # then remaining batch chunks.
            w_sb = [[None] * _NJ for _ in range(_NK)]
            xh_sb = [[None] * _NB for _ in range(_NK)]

            def load_w(k, j):
                t_ = wp.tile([128, _NT * 128], f32r, tag=f"w{k}_{j}")
                nc.sync.dma_start(
                    out=t_[:], in_=wT[k * 128:(k + 1) * 128,
                                      j * 512:(j + 1) * 512])
                w_sb[k][j] = t_

            def load_xh(k, bc):
                t_ = xp.tile([128, _BCH], f32r, tag=f"xh{k}_{bc}")
                src = xT if k < _NK // 2 else hT
                r = (k % (_NK // 2)) * 128
                nc.sync.dma_start(
                    out=t_[:], in_=src[r:r + 128,
                                       bc * _BCH:(bc + 1) * _BCH])
                xh_sb[k][bc] = t_

            for k in range(_NK):
                load_w(k, 0)
                load_xh(k, 0)
            for j in range(1, _NJ):
                for k in range(_NK):
                    load_w(k, j)
            for bc in range(1, _NB):
                for k in range(_NK):
                    load_xh(k, bc)

            for bc in range(_NB):
                bsl = slice(bc * _BCH, (bc + 1) * _BCH)
                for j in range(_NJ):
                    ps = []
                    for t in range(_NT):
                        pstile = pp.tile([128, _BCH], f32, tag=f"ps{t}")
                        for k in range(_NK):
                            nc.tensor.matmul(
                                pstile[:],
                                w_sb[k][j][:, t * 128:(t + 1) * 128],
                                xh_sb[k][bc][:],
                                start=(k == 0), stop=(k == _NK - 1),
                            )
                        ps.append(pstile)
                    c_sb = cp.tile([128, _BCH], f32, tag="c")
                    nc.scalar.dma_start(out=c_sb[:],
                                        in_=cT[j * 128:(j + 1) * 128, bsl])
                    gI = ap.tile([128, _BCH], f32, tag="gI")
                    gF = ap.tile([128, _BCH], f32, tag="gF")
                    gG = ap.tile([128, _BCH], f32, tag="gG")
                    gO = ap.tile([128, _BCH], f32, tag="gO")
                    bcol = j * _NT
                    nc.scalar.activation(gI[:], ps[0][:], AF.Sigmoid,
                                         bias=bias_sb[:, bcol + 0:bcol + 1])
                    nc.scalar.activation(gF[:], ps[1][:], AF.Sigmoid,
                                         bias=bias_sb[:, bcol + 1:bcol + 2])
                    nc.scalar.activation(gG[:], ps[2][:], AF.Tanh,
                                         bias=bias_sb[:, bcol + 2:bcol + 3])
                    nc.scalar.activation(gO[:], ps[3][:], AF.Sigmoid,
                                         bias=bias_sb[:, bcol + 3:bcol + 4])
                    newc = op.tile([128, _BCH], f32, tag="newc")
                    newh = op.tile([128, _BCH], f32, tag="newh")
                    nc.vector.tensor_mul(gF[:], gF[:], c_sb[:])   # f * c
                    nc.vector.tensor_mul(gI[:], gI[:], gG[:])     # i * g
                    nc.vector.tensor_add(newc[:], gF[:], gI[:])
                    nc.scalar.activation(gG[:], newc[:], AF.Tanh)
                    nc.vector.tensor_mul(newh[:], gO[:], gG[:])
                    nc.scalar.dma_start(out=coT[j * 128:(j + 1) * 128, bsl],
                                        in_=newc[:])
                    nc.scalar.dma_start(out=hoT[j * 128:(j + 1) * 128, bsl],
                                        in_=newh[:])

        if reps == 1:
            body()
        else:
            with tc.For_i(0, reps, 1):
                body()
    nc.compile()
    return nc


# Gate-dim permutation: position j*4 + t  <-  original gate tile t*4 + j
# (tile index into the stacked-gates dim of 16 x 128 rows).
def _gate_perm():
    perm = np.empty(_G, np.int64)
    pos = 0
    for j in range(_NJ):
        for t in range(_NT):
            src = (t * _NJ + j) * 128
            perm[pos:pos + 128] = np.arange(src, src + 128)
            pos += 128
    return perm


def _host_shards(x, h, c, Wi, bi, Wh, bh):
    perm = _gate_perm()
    W = np.concatenate([np.asarray(Wi, np.float32),
                        np.asarray(Wh, np.float32)], axis=1)    # [G, K]
    wTv = np.ascontiguousarray(W[perm].T)                        # [K, G] permuted
    b = (np.asarray(bi, np.float32) + np.asarray(bh, np.float32))[perm]
    b2 = np.ascontiguousarray(b.reshape(_G // 128, 128).T)       # [128, G/128]
    in_maps = []
    for s in range(_NC):
        sl = slice(s * _BL, (s + 1) * _BL)
        in_maps.append({
            "xT": np.ascontiguousarray(np.asarray(x, np.float32)[sl].T),
            "hT": np.ascontiguousarray(np.asarray(h, np.float32)[sl].T),
            "cT": np.ascontiguousarray(np.asarray(c, np.float32)[sl].T),
            "wT": wTv,
            "b2": b2,
        })
    return in_maps


def kernel(x, h, c, Wi, bi, Wh, bh):
    from concourse.bass_utils import run_bass_kernel_spmd

    nc = _cache.get("nc")
    if nc is None:
        nc = _build()
        _cache["nc"] = nc

    in_maps = _host_shards(x, h, c, Wi, bi, Wh, bh)
    res = run_bass_kernel_spmd(nc, in_maps, list(range(_NC)))

    h_out = np.empty((_B, _H), np.float32)
    c_out = np.empty((_B, _H), np.float32)
    for s in range(_NC):
        sl = slice(s * _BL, (s + 1) * _BL)
        h_out[sl] = res.results[s]["hoT"].T
        c_out[sl] = res.results[s]["coT"].T
    return h_out, c_out



# revision 4
# speedup vs baseline: 1.2558x; 1.2558x over previous
"""LSTMCell (B=16384, I=H=512) on 8 Trainium2 NeuronCores.

Strategy: data-parallel over the batch (2048 rows/core). Each core computes
gatesT = W @ [x;h]T in transposed layout (gate dim on partitions, batch on the
free dim) so that:
  - the contraction dim (I+H) lands on SBUF partitions for both matmul
    operands with zero on-chip transposes (inputs are pre-transposed on the
    host while sharding),
  - the gate bias is a per-partition vector, applied for free by the ScalarE
    activation instruction.
All matmul operands are bf16 (host-cast; rel err ~1e-3 vs the 2e-2 budget),
which runs the PE at the same rate as fp32r but halves HBM traffic and SBUF
footprint. That headroom buys full double-buffering: weights and activations
for rep N+1 stream while rep N computes, in 512KB DMAs ([128, 2048] per
contraction slice, x and h fused into one xhT tensor).
The stacked gate dim is permuted on the host so each 128-row h-block's four
gate tiles (i, f, g, o) are contiguous in the weight matrix.
Elementwise LSTM tail (sigmoid/tanh/mul/add) runs on ScalarE + VectorE
overlapped with the matmuls; c loads and bf16 result stores ride the scalar
HWDGE ring so they never queue behind the big sync-ring loads. Outputs are
stored transposed in bf16 and un-transposed/upcast on the host.
"""

import numpy as np
from contextlib import ExitStack

_B, _I, _H = 16384, 512, 512
_NC = 8
_BL = _B // _NC          # 2048 batch rows per core
_G = 4 * _H              # 2048 stacked gate dim
_K = _I + _H             # 1024 contraction dim
_BCH = 512               # batch chunk (PSUM bank free size)
_NB = _BL // _BCH        # 4 batch chunks
_NJ = _H // 128          # 4 h-blocks of 128
_NK = _K // 128          # 8 k-chunks of 128
_NT = 4                  # gates (i, f, g, o)

_cache = {}


def _build(reps=1, unroll=False):
    from concourse import bacc
    import concourse.mybir as mybir
    import concourse.tile as tile

    f32 = mybir.dt.float32
    bf16 = mybir.dt.bfloat16
    AF = mybir.ActivationFunctionType

    nc = bacc.Bacc("TRN2", target_bir_lowering=False, debug=False,
                   num_devices=_NC)
    xhT = nc.declare_dram_parameter("xhT", [_K, _BL], bf16, isOutput=False)
    cT = nc.declare_dram_parameter("cT", [_H, _BL], bf16, isOutput=False)
    # gate dim pre-permuted on host: column block j*512..j*512+512 holds the
    # (i, f, g, o) tiles for h-block j, each 128 wide.
    wT = nc.declare_dram_parameter("wT", [_K, _G], bf16, isOutput=False)
    b2 = nc.declare_dram_parameter("b2", [128, _G // 128], f32, isOutput=False)
    hoT = nc.declare_dram_parameter("hoT", [_H, _BL], bf16, isOutput=True)
    coT = nc.declare_dram_parameter("coT", [_H, _BL], bf16, isOutput=True)

    with ExitStack() as ctx:
        tc = ctx.enter_context(tile.TileContext(nc))
        wp = ctx.enter_context(tc.tile_pool(name="w", bufs=2))
        xp = ctx.enter_context(tc.tile_pool(name="xh", bufs=2))
        bp = ctx.enter_context(tc.tile_pool(name="bias", bufs=1))
        cp = ctx.enter_context(tc.tile_pool(name="cin", bufs=2))
        ap = ctx.enter_context(tc.tile_pool(name="act", bufs=2))
        op = ctx.enter_context(tc.tile_pool(name="out", bufs=2))
        pp = ctx.enter_context(tc.tile_pool(name="ps", bufs=2, space="PSUM"))

        def body(_iv=None):
            bias_sb = bp.tile([128, _G // 128], f32, tag="bias")
            nc.sync.dma_start(out=bias_sb[:], in_=b2[:])

            # One 512KB DMA per contraction slice k: weights cover all
            # (j, t) blocks, activations cover all batch chunks. Interleaved
            # so the sync HWDGE ring delivers matched (w, xh) pairs in the
            # order the PE consumes them.
            w_sb = [None] * _NK
            xh_sb = [None] * _NK
            for k in range(_NK):
                wt = wp.tile([128, _G], bf16, tag=f"w{k}")
                nc.sync.dma_start(out=wt[:],
                                  in_=wT[k * 128:(k + 1) * 128, :])
                w_sb[k] = wt
                xt = xp.tile([128, _BL], bf16, tag=f"xh{k}")
                nc.sync.dma_start(out=xt[:],
                                  in_=xhT[k * 128:(k + 1) * 128, :])
                xh_sb[k] = xt

            # c input per h-block, all batch chunks in one 256KB DMA, on the
            # scalar ring.
            c_sb = [None] * _NJ
            for j in range(_NJ):
                ct = cp.tile([128, _BL], bf16, tag=f"c{j}")
                nc.scalar.dma_start(out=ct[:],
                                    in_=cT[j * 128:(j + 1) * 128, :])
                c_sb[j] = ct

            for bc in range(_NB):
                bsl = slice(bc * _BCH, (bc + 1) * _BCH)
                for j in range(_NJ):
                    ps = []
                    for t in range(_NT):
                        pstile = pp.tile([128, _BCH], f32, tag=f"ps{t}")
                        wcol = j * 512 + t * 128
                        for k in range(_NK):
                            nc.tensor.matmul(
                                pstile[:],
                                w_sb[k][:, wcol:wcol + 128],
                                xh_sb[k][:, bsl],
                                start=(k == 0), stop=(k == _NK - 1),
                            )
                        ps.append(pstile)
                    gI = ap.tile([128, _BCH], f32, tag="gI")
                    gF = ap.tile([128, _BCH], f32, tag="gF")
                    gG = ap.tile([128, _BCH], f32, tag="gG")
                    gO = ap.tile([128, _BCH], f32, tag="gO")
                    bcol = j * _NT
                    nc.scalar.activation(gI[:], ps[0][:], AF.Sigmoid,
                                         bias=bias_sb[:, bcol + 0:bcol + 1])
                    nc.scalar.activation(gF[:], ps[1][:], AF.Sigmoid,
                                         bias=bias_sb[:, bcol + 1:bcol + 2])
                    nc.scalar.activation(gG[:], ps[2][:], AF.Tanh,
                                         bias=bias_sb[:, bcol + 2:bcol + 3])
                    nc.scalar.activation(gO[:], ps[3][:], AF.Sigmoid,
                                         bias=bias_sb[:, bcol + 3:bcol + 4])
                    newc = op.tile([128, _BCH], bf16, tag="newc")
                    newh = op.tile([128, _BCH], bf16, tag="newh")
                    nc.vector.tensor_mul(gF[:], gF[:], c_sb[j][:, bsl])
                    nc.vector.tensor_mul(gI[:], gI[:], gG[:])     # i * g
                    nc.vector.tensor_add(newc[:], gF[:], gI[:])
                    nc.scalar.activation(gG[:], newc[:], AF.Tanh)
                    nc.vector.tensor_mul(newh[:], gO[:], gG[:])
                    nc.scalar.dma_start(out=coT[j * 128:(j + 1) * 128, bsl],
                                        in_=newc[:])
                    nc.scalar.dma_start(out=hoT[j * 128:(j + 1) * 128, bsl],
                                        in_=newh[:])

        if reps == 1:
            body()
        elif unroll:
            for _ in range(reps):
                body()
        else:
            with tc.For_i(0, reps, 1):
                body()
    nc.compile()
    return nc


# Gate-dim permutation: position j*4 + t  <-  original gate tile t*4 + j
# (tile index into the stacked-gates dim of 16 x 128 rows).
def _gate_perm():
    perm = np.empty(_G, np.int64)
    pos = 0
    for j in range(_NJ):
        for t in range(_NT):
            src = (t * _NJ + j) * 128
            perm[pos:pos + 128] = np.arange(src, src + 128)
            pos += 128
    return perm


def _bf16():
    import ml_dtypes
    return ml_dtypes.bfloat16


def _host_shards(x, h, c, Wi, bi, Wh, bh):
    bf16 = _bf16()
    perm = _gate_perm()
    W = np.concatenate([np.asarray(Wi, np.float32),
                        np.asarray(Wh, np.float32)], axis=1)    # [G, K]
    wTv = np.ascontiguousarray(W[perm].T.astype(bf16))          # [K, G]
    b = (np.asarray(bi, np.float32) + np.asarray(bh, np.float32))[perm]
    b2 = np.ascontiguousarray(b.reshape(_G // 128, 128).T)      # [128, G/128]
    xh = np.concatenate([np.asarray(x, np.float32),
                         np.asarray(h, np.float32)], axis=1)    # [B, K]
    in_maps = []
    for s in range(_NC):
        sl = slice(s * _BL, (s + 1) * _BL)
        in_maps.append({
            "xhT": np.ascontiguousarray(xh[sl].T.astype(bf16)),
            "cT": np.ascontiguousarray(
                np.asarray(c, np.float32)[sl].T.astype(bf16)),
            "wT": wTv,
            "b2": b2,
        })
    return in_maps


def kernel(x, h, c, Wi, bi, Wh, bh):
    from concourse.bass_utils import run_bass_kernel_spmd

    nc = _cache.get("nc")
    if nc is None:
        nc = _build()
        _cache["nc"] = nc

    in_maps = _host_shards(x, h, c, Wi, bi, Wh, bh)
    res = run_bass_kernel_spmd(nc, in_maps, list(range(_NC)))

    h_out = np.empty((_B, _H), np.float32)
    c_out = np.empty((_B, _H), np.float32)
    for s in range(_NC):
        sl = slice(s * _BL, (s + 1) * _BL)
        h_out[sl] = res.results[s]["hoT"].astype(np.float32).T
        c_out[sl] = res.results[s]["coT"].astype(np.float32).T
    return h_out, c_out


# revision 17
# speedup vs baseline: 1.4138x; 1.1258x over previous
"""LSTMCell (B=16384, I=H=512) on 8 Trainium2 NeuronCores.

Strategy: data-parallel over the batch (2048 rows/core). Each core computes
gatesT = W @ [x;h]T in transposed layout (gate dim on partitions, batch on the
free dim) so that:
  - the contraction dim (I+H) lands on SBUF partitions for both matmul
    operands with zero on-chip transposes (inputs are pre-transposed on the
    host while sharding),
  - the gate bias is a per-partition vector, applied for free by the ScalarE
    activation instruction.
All matmul operands are bf16 (host-cast; rel err ~1e-3 vs the 2e-2 budget),
which runs the PE at the same rate as fp32r but halves HBM traffic and SBUF
footprint. That headroom buys full double-buffering: weights and activations
for rep N+1 stream while rep N computes, in 512KB DMAs ([128, 2048] per
contraction slice, x and h fused into one xhT tensor).
The stacked gate dim is permuted on the host so each 128-row h-block's four
gate tiles (i, f, g, o) are contiguous in the weight matrix.
Elementwise LSTM tail (sigmoid/tanh/mul/add) runs on ScalarE + VectorE
overlapped with the matmuls; c loads and bf16 result stores ride the scalar
HWDGE ring so they never queue behind the big sync-ring loads. Outputs are
stored transposed in bf16 and un-transposed/upcast on the host.
"""

import numpy as np
from contextlib import ExitStack

_B, _I, _H = 16384, 512, 512
_NC = 8
_BL = _B // _NC          # 2048 batch rows per core
_G = 4 * _H              # 2048 stacked gate dim
_K = _I + _H             # 1024 contraction dim
_BCH = 512               # batch chunk (PSUM bank free size)
_NB = _BL // _BCH        # 4 batch chunks
_NJ = _H // 128          # 4 h-blocks of 128
_NK = _K // 128          # 8 k-chunks of 128
_NT = 4                  # gates (i, f, g, o)

_cache = {}


def _build(reps=1, unroll=False):
    from concourse import bacc
    import concourse.mybir as mybir
    import concourse.tile as tile

    f32 = mybir.dt.float32
    bf16 = mybir.dt.bfloat16
    AF = mybir.ActivationFunctionType

    nc = bacc.Bacc("TRN2", target_bir_lowering=False, debug=False,
                   num_devices=_NC)
    xhT = nc.declare_dram_parameter("xhT", [_K, _BL], bf16, isOutput=False)
    cT = nc.declare_dram_parameter("cT", [_H, _BL], bf16, isOutput=False)
    # gate dim pre-permuted on host: column block j*512..j*512+512 holds the
    # (i, f, g, o) tiles for h-block j, each 128 wide.
    wT = nc.declare_dram_parameter("wT", [_K, _G], bf16, isOutput=False)
    b2 = nc.declare_dram_parameter("b2", [128, _G // 128], f32, isOutput=False)
    hoT = nc.declare_dram_parameter("hoT", [_H, _BL], bf16, isOutput=True)
    coT = nc.declare_dram_parameter("coT", [_H, _BL], bf16, isOutput=True)

    with ExitStack() as ctx:
        tc = ctx.enter_context(tile.TileContext(nc))
        wp = ctx.enter_context(tc.tile_pool(name="w", bufs=2))
        xp = ctx.enter_context(tc.tile_pool(name="xh", bufs=2))
        bp = ctx.enter_context(tc.tile_pool(name="bias", bufs=2))
        cp = ctx.enter_context(tc.tile_pool(name="cin", bufs=2))
        ap = ctx.enter_context(tc.tile_pool(name="act", bufs=2))
        op = ctx.enter_context(tc.tile_pool(name="out", bufs=2))
        pp = ctx.enter_context(tc.tile_pool(name="ps", bufs=2, space="PSUM"))

        def alloc_set():
            s = {"w": [None] * _NK, "xh": [None] * _NK, "c": [None] * _NJ}
            for k in range(_NK):
                s["w"][k] = wp.tile([128, _G], bf16, tag=f"w{k}",
                                    name=f"w{k}")
                s["xh"][k] = xp.tile([128, _BL], bf16, tag=f"xh{k}",
                                     name=f"xh{k}")
            for j in range(_NJ):
                s["c"][j] = cp.tile([128, _BL], bf16, tag=f"c{j}",
                                    name=f"c{j}")
            s["bias"] = bp.tile([128, _G // 128], f32, tag="bias",
                                name="bias")
            return s

        def load_set(s):
            # One 512KB DMA per contraction slice k on the sync HWDGE ring:
            # weights cover all (j, t) blocks, activations all batch chunks.
            # c rides the scalar ring with the result stores.
            for k in range(_NK):
                nc.sync.dma_start(out=s["w"][k][:],
                                  in_=wT[k * 128:(k + 1) * 128, :])
                nc.sync.dma_start(out=s["xh"][k][:],
                                  in_=xhT[k * 128:(k + 1) * 128, :])
            for j in range(_NJ):
                nc.scalar.dma_start(out=s["c"][j][:],
                                    in_=cT[j * 128:(j + 1) * 128, :])
            nc.sync.dma_start(out=s["bias"][:], in_=b2[:])

        def compute(s):
            # (j, t) outer / bc inner: each 128x128 weight tile is stationary
            # for 4 consecutive matmuls (the 4 batch chunks), quartering the
            # LDWEIGHTS traffic on the PE. PSUM: one bank per bc, x2 buffers;
            # the ScalarE activations drain each bank while the PE streams
            # the next group, keeping the PE at its 2.4GHz issue rate.
            w_sb, xh_sb, c_sb, bias_sb = s["w"], s["xh"], s["c"], s["bias"]
            AFS = [AF.Sigmoid, AF.Sigmoid, AF.Tanh, AF.Sigmoid]
            for j in range(_NJ):
                gt = [[None] * _NB for _ in range(_NT)]
                for t in range(_NT):
                    ps = []
                    for bc in range(_NB):
                        pst = pp.tile([128, _BCH], f32, tag=f"ps{bc}")
                        ps.append(pst)
                    wcol = j * 512 + t * 128
                    for k in range(_NK):
                        for bc in range(_NB):
                            nc.tensor.matmul(
                                ps[bc][:],
                                w_sb[k][:, wcol:wcol + 128],
                                xh_sb[k][:, bc * _BCH:(bc + 1) * _BCH],
                                start=(k == 0), stop=(k == _NK - 1),
                            )
                    bias_ap = bias_sb[:, j * _NT + t:j * _NT + t + 1]
                    for bc in range(_NB):
                        g_ = ap.tile([128, _BCH], bf16, tag=f"g{t}_{bc}")
                        nc.scalar.activation(g_[:], ps[bc][:], AFS[t],
                                             bias=bias_ap)
                        gt[t][bc] = g_
                for bc in range(_NB):
                    bsl = slice(bc * _BCH, (bc + 1) * _BCH)
                    gI, gF, gG, gO = (gt[0][bc], gt[1][bc],
                                      gt[2][bc], gt[3][bc])
                    fc = op.tile([128, _BCH], bf16, tag="fc")
                    ig = op.tile([128, _BCH], bf16, tag="ig")
                    newc = op.tile([128, _BCH], bf16, tag="newc")
                    newh = op.tile([128, _BCH], bf16, tag="newh")
                    nc.vector.tensor_mul(fc[:], gF[:], c_sb[j][:, bsl])
                    nc.vector.tensor_mul(ig[:], gI[:], gG[:])     # i * g
                    nc.vector.tensor_add(newc[:], fc[:], ig[:])
                    nc.scalar.activation(gG[:], newc[:], AF.Tanh)
                    nc.vector.tensor_mul(newh[:], gO[:], gG[:])
                    nc.scalar.dma_start(out=coT[j * 128:(j + 1) * 128, bsl],
                                        in_=newc[:])
                    nc.scalar.dma_start(out=hoT[j * 128:(j + 1) * 128, bsl],
                                        in_=newh[:])

        if reps == 1:
            sA = alloc_set()
            load_set(sA)
            compute(sA)
        else:
            # Software pipeline: two resident input sets; each loop iteration
            # runs two reps, loading one set's next inputs while computing
            # from the other. The For_i back-edge all-engine barrier
            # (~2-4us) is amortized over two reps and never sits between a
            # load and its consumer.
            assert reps % 2 == 0, "pipelined timing build needs even reps"
            sA = alloc_set()
            sB = alloc_set()
            load_set(sA)

            def body(_iv=None):
                load_set(sB)
                compute(sA)
                load_set(sA)
                compute(sB)

            if unroll:
                for _ in range(reps // 2):
                    body()
            else:
                engines = tuple(mybir.ALL_ENGINES)
                with tc.For_i(0, reps // 2, 1, hint_engines=engines):
                    body()
    nc.compile()
    return nc


# Gate-dim permutation: position j*4 + t  <-  original gate tile t*4 + j
# (tile index into the stacked-gates dim of 16 x 128 rows).
def _gate_perm():
    perm = np.empty(_G, np.int64)
    pos = 0
    for j in range(_NJ):
        for t in range(_NT):
            src = (t * _NJ + j) * 128
            perm[pos:pos + 128] = np.arange(src, src + 128)
            pos += 128
    return perm


def _bf16():
    import ml_dtypes
    return ml_dtypes.bfloat16


def _host_shards(x, h, c, Wi, bi, Wh, bh):
    bf16 = _bf16()
    perm = _gate_perm()
    W = np.concatenate([np.asarray(Wi, np.float32),
                        np.asarray(Wh, np.float32)], axis=1)    # [G, K]
    wTv = np.ascontiguousarray(W[perm].T.astype(bf16))          # [K, G]
    b = (np.asarray(bi, np.float32) + np.asarray(bh, np.float32))[perm]
    b2 = np.ascontiguousarray(b.reshape(_G // 128, 128).T)      # [128, G/128]
    xh = np.concatenate([np.asarray(x, np.float32),
                         np.asarray(h, np.float32)], axis=1)    # [B, K]
    in_maps = []
    for s in range(_NC):
        sl = slice(s * _BL, (s + 1) * _BL)
        in_maps.append({
            "xhT": np.ascontiguousarray(xh[sl].T.astype(bf16)),
            "cT": np.ascontiguousarray(
                np.asarray(c, np.float32)[sl].T.astype(bf16)),
            "wT": wTv,
            "b2": b2,
        })
    return in_maps


def kernel(x, h, c, Wi, bi, Wh, bh):
    from concourse.bass_utils import run_bass_kernel_spmd

    nc = _cache.get("nc")
    if nc is None:
        nc = _build()
        _cache["nc"] = nc

    in_maps = _host_shards(x, h, c, Wi, bi, Wh, bh)
    res = run_bass_kernel_spmd(nc, in_maps, list(range(_NC)))

    h_out = np.empty((_B, _H), np.float32)
    c_out = np.empty((_B, _H), np.float32)
    for s in range(_NC):
        sl = slice(s * _BL, (s + 1) * _BL)
        h_out[sl] = res.results[s]["hoT"].astype(np.float32).T
        c_out[sl] = res.results[s]["coT"].astype(np.float32).T
    return h_out, c_out
